# revision 2
# baseline (speedup 1.0000x reference)
"""TRN2 Bass kernel for GNN message passing (nn_MessagePassing):

    out = segment_sum(x[src] * edge_weight, dst, num_segments=N)

x: [50000, 64] f32, edge_weight: [1250000] f32, edge_index: [2, 1250000] i64.

Distribution strategy (8 NeuronCores, SPMD):
  - Destination nodes are sharded across the 8 cores (core k owns output rows
    [k*6250, (k+1)*6250)), so no all-reduce is needed: each core computes a
    disjoint output slice and the host concatenates them.
  - x is pair-packed to bf16 on the host: HBM table row p holds
    concat(x[2p], x[2p+1]) as [25000, 128] bf16 (256B rows).  Pair ids fit
    int16, so there is no table split and gather calls run contiguously.

Per-core device pipeline:
  - Host buckets edges by (core, 64-node dst window, src parity) and pads
    each bucket to a multiple of 128 edges ("chunks").
  - dma_gather bulk-gathers the source row-pairs for up to 8 chunks
    (1024 idx) per instruction from the bf16 pair table.
  - The weighted one-hot scatter matrix S_w (S_w[e, j] = w_e*[dst_local==j])
    is built by the HOST and streamed from HBM as bf16 — the vector engine
    does no per-edge work at all.
  - out_window += S_w_chunk^T @ msg_half accumulates in PSUM on the tensor
    engine fully in bf16 (1 PE cycle/row vs 4 for fp32); the matmul rhs
    selects the correct 64-column half of each gathered pair per bucket.
  - Finished windows are copied to SBUF on the Activation engine and DMA'd
    to the output.
"""

import sys

if "/opt/trn_rl_repo" not in sys.path:
    sys.path.insert(0, "/opt/trn_rl_repo")

import numpy as np
from ml_dtypes import bfloat16

import concourse.bacc as bacc
import concourse.mybir as mybir
import concourse.tile as tile
from concourse.bass_utils import run_bass_kernel_spmd

N_CORES = 8
F = 64
W = 64            # dst-window width (one-hot matmul M dim)
CHUNK = 128       # edges per matmul (K dim)
GROUP_CHUNKS = 48  # max chunks per window-group (SBUF budget)
NPAIR = 25000     # x row-pairs


def _cdiv(a, b):
    return (a + b - 1) // b


def _host_prep(x, edge_weight, edge_index):
    N = x.shape[0]
    npc = N // N_CORES
    nw = _cdiv(npc, W)
    src = np.asarray(edge_index[0]).astype(np.int64)
    dst = np.asarray(edge_index[1]).astype(np.int64)
    wgt = np.asarray(edge_weight).astype(np.float32)
    E = src.shape[0]

    core = dst // npc
    rel = dst - core * npc
    win = rel // W
    dstl = rel % W
    par = src & 1
    key = (core * nw + win) * 2 + par
    order = np.argsort(key, kind="stable")
    key_s = key[order]
    src_s = src[order]
    dstl_s = dstl[order]
    wgt_s = wgt[order]

    nbuckets = N_CORES * nw * 2
    counts = np.bincount(key_s, minlength=nbuckets).reshape(N_CORES, nw, 2)
    nchunks = _cdiv(counts, CHUNK).max(axis=0)      # [nw, 2], SPMD-uniform
    nchunks[:, 0] = np.maximum(nchunks[:, 0], 1)

    groups = []
    w0, acc = 0, 0
    for w in range(nw):
        t = int(nchunks[w].sum())
        if acc + t > GROUP_CHUNKS and acc > 0:
            groups.append((w0, w))
            w0, acc = w, 0
        acc += t
    groups.append((w0, nw))

    col0 = np.zeros((nw, 2), np.int64)
    gmeta = []
    c = 0
    for (ws, we) in groups:
        c0 = c
        cc = c0
        for w in range(ws, we):
            for p in (0, 1):
                col0[w, p] = cc
                cc += nchunks[w, p]
        c = cc
        gmeta.append((c0, c - c0, ws, we))
    K_PAD = c

    bstart = np.concatenate([[0], np.cumsum(counts.reshape(-1))])
    slot_base = np.zeros(nbuckets, np.int64)
    for ci in range(N_CORES):
        for w in range(nw):
            for p in (0, 1):
                slot_base[(ci * nw + w) * 2 + p] = col0[w, p] * CHUNK
    rank = np.arange(E) - bstart[key_s]
    slot = slot_base[key_s] + rank

    x32 = np.asarray(x, dtype=np.float32)
    xp = np.ascontiguousarray(x32.astype(bfloat16).reshape(NPAIR, 2 * F))

    in_maps = []
    for ci in range(N_CORES):
        lo = np.searchsorted(key_s, ci * nw * 2, "left")
        hi_ = np.searchsorted(key_s, (ci + 1) * nw * 2, "left")
        sl = slot[lo:hi_]
        idx_stream = np.zeros(K_PAD * CHUNK, np.int32)
        idx_stream[sl] = src_s[lo:hi_] >> 1
        idx16 = np.tile(
            idx_stream.astype(np.int16).reshape(K_PAD * 8, 16).T, (8, 1))

        # host-built weighted one-hot scatter matrix, streamed as bf16:
        # S[e_in_chunk, chunk*W + dstl] = wgt
        S = np.zeros((128, K_PAD * W), np.float32)
        e_in_chunk = (sl % CHUNK).astype(np.int64)
        chunk_id = sl // CHUNK
        S[e_in_chunk, chunk_id * W + dstl_s[lo:hi_]] = wgt_s[lo:hi_]
        in_maps.append({
            "xp": xp,
            "idx16": np.ascontiguousarray(idx16),
            "sw": S.astype(bfloat16),
        })

    meta = dict(N=N, npc=npc, nw=nw, K_PAD=K_PAD,
                nchunks=nchunks, col0=col0, gmeta=gmeta)
    return in_maps, meta


def _build_program(meta, reps=1, msg_bufs=5, s_bufs=5, gather_chunks=8):
    npc, nw, K_PAD = meta["npc"], meta["nw"], meta["K_PAD"]
    nchunks, col0 = meta["nchunks"], meta["col0"]
    gmeta = meta["gmeta"]
    f32, bf16, i16 = mybir.dt.float32, mybir.dt.bfloat16, mybir.dt.int16

    nc = bacc.Bacc("TRN2", target_bir_lowering=False, debug=False,
                   num_devices=N_CORES, num_swdge_queues=4)
    xp_d = nc.dram_tensor("xp", [NPAIR, 2 * F], bf16, kind="ExternalInput")
    idx_d = nc.dram_tensor("idx16", [128, K_PAD * 8], i16,
                           kind="ExternalInput")
    sw_d = nc.dram_tensor("sw", [128, K_PAD * W], bf16, kind="ExternalInput")
    out_d = nc.dram_tensor("out", [npc, F], f32, kind="ExternalOutput")

    with tile.TileContext(nc) as tc:
        with (
            tc.tile_pool(name="aux", bufs=1) as aux,
            tc.tile_pool(name="big", bufs=msg_bufs) as big,
            tc.tile_pool(name="opool", bufs=2) as opool,
            tc.tile_pool(name="psum", bufs=8, space="PSUM") as pp,
        ):
            idx_t = aux.tile([128, K_PAD * 8], i16)
            nc.sync.dma_start(out=idx_t[:], in_=idx_d.ap()[:])

            qctr = [0]

            def body():
                o_t = opool.tile([W, nw * F], f32, tag="obig")
                for (c0, kg, ws, we) in gmeta:
                    msg_t = big.tile([128, kg * 2 * F], bf16, tag="msg")
                    done = 0
                    while done < kg:
                        blk = min(gather_chunks, kg - done)
                        cg = c0 + done
                        nc.gpsimd.dma_gather(
                            out_ap=msg_t[:, done * 2 * F:
                                         (done + blk) * 2 * F].rearrange(
                                "p (c f) -> p c f", f=2 * F),
                            in_ap=xp_d.ap()[:],
                            idxs_ap=idx_t[:, cg * 8:(cg + blk) * 8],
                            num_idxs=blk * CHUNK,
                            num_idxs_reg=blk * CHUNK,
                            elem_size=2 * F,
                            queue_num=qctr[0] % 4,
                        )
                        qctr[0] += 1
                        done += blk
                    S_t = big.tile([128, kg * W], bf16, tag="S", bufs=s_bufs)
                    nc.sync.dma_start(
                        out=S_t[:], in_=sw_d.ap()[:, c0 * W:(c0 + kg) * W])
                    for w in range(ws, we):
                        cols = []
                        for p in (0, 1):
                            for j in range(int(nchunks[w, p])):
                                cols.append((int(col0[w, p]) + j, p))
                        ps = pp.tile([W, F], f32, tag="ps")
                        for j, (cg, p) in enumerate(cols):
                            cc = cg - c0
                            nc.tensor.matmul(
                                out=ps[:],
                                lhsT=S_t[:, cc * W:(cc + 1) * W],
                                rhs=msg_t[:, cc * 2 * F + p * F:
                                          cc * 2 * F + p * F + F],
                                start=(j == 0), stop=(j == len(cols) - 1))
                        nc.scalar.activation(
                            out=o_t[:, w * F:(w + 1) * F], in_=ps[:],
                            func=mybir.ActivationFunctionType.Copy)
                    for w in range(ws, we):
                        rows = min(W, npc - w * W)
                        nc.sync.dma_start(
                            out=out_d.ap()[w * W:w * W + rows, :],
                            in_=o_t[:rows, w * F:(w + 1) * F])

            for _ in range(reps):
                body()
    nc.compile()
    return nc


def build_for_inputs(x, edge_weight, edge_index, reps=1, **knobs):
    """Exposed for test harnesses: returns (nc, in_maps, meta)."""
    in_maps, meta = _host_prep(x, edge_weight, edge_index)
    nc = _build_program(meta, reps=reps, **knobs)
    return nc, in_maps, meta


def kernel(x, edge_weight, edge_index):
    x = np.asarray(x)
    nc, in_maps, _meta = build_for_inputs(x, edge_weight, edge_index)
    res = run_bass_kernel_spmd(nc, in_maps, core_ids=list(range(N_CORES)))
    out = np.concatenate(
        [res.results[c]["out"] for c in range(N_CORES)], axis=0)
    return out.astype(np.float32)
